# revision 2
# baseline (speedup 1.0000x reference)
"""MoE (8 experts, top-2, cap-drop) Trainium2 kernel over 8 NeuronCores.

Strategy (expert-parallel, per sharding hint):
 - Router runs replicated on host (tiny: 134 MFLOP of the 344 GFLOP total) with
   the exact fp32 jax ops of the reference so top-2/capacity decisions match
   the oracle bit-for-bit; routing IS the sharding function here - it decides
   which token rows go to which expert shard.
 - Dispatch/shard: per expert e, gather its routed token rows (cap=2560 slots,
   ascending token order, gate 0 on padding slots) and ship them transposed
   (D on partitions) to the cores owning expert e.
 - Each expert's FFN is split across 2 cores along the hidden (FFN) axis:
   core pair (2j, 2j+1) serves experts {2j, 2j+1}; core 2j computes hidden
   units [0,2048), core 2j+1 units [2048,4096). Each core therefore runs two
   sequential "units" (one per expert) of a dense double matmul:
       ysT[u] = W2u^T-chain( relu( W1u^T-chain( xgT ) ) )
   with all matmuls in f32r (TF32 storage class, fp32 PSUM accumulate) at
   1 cycle/row on the PE array.
 - Combine/unshard: host sums the two half partial outputs per expert, applies
   the fp32 gates exactly as the reference (g[:,None] * h), and scatter-adds
   into y at the routed rows (row sets are disjoint within an expert).

Self-contained: shapes/sharding hardcoded for B=4, S=2048, D=1024, F=4096,
E=8, top-2, cap=2560, 8 cores.
"""

import sys

for _p in ("/opt/trn_rl_repo",):
    if _p not in sys.path:
        sys.path.append(_p)

import math

import numpy as np

B, S, D, F, E = 4, 2048, 1024, 4096, 8
TOP_K = 2
CAP_FACTOR = 1.25
T = B * S                                   # 8192 tokens
CAP = max(math.ceil(T * TOP_K * CAP_FACTOR / E), 1)   # 2560
FH = F // 2                                 # 2048 hidden units per core
NCORES = 8
BLK = 512                                   # token block (matmul moving dim)
NBLK = CAP // BLK                           # 5
P = 128


def _tf32_round(a: np.ndarray) -> np.ndarray:
    """Round-to-nearest-even fp32 -> tf32 (10-bit mantissa) on the host, so the
    on-device f32r matmuls see exactly-representable values (measured exact)."""
    u = a.reshape(-1).view(np.uint32).astype(np.uint64)
    r = (u + 0xFFF + ((u >> 13) & 1)) & ~np.uint64(0x1FFF)
    return r.astype(np.uint32).view(np.float32).reshape(a.shape)


def _route(x: np.ndarray, Wr: np.ndarray):
    """Replicate the reference's routing bit-for-bit on jax-CPU.

    Returns per-expert (idx[CAP] int32 token ids, gate[CAP] f32, 0 on padding).
    """
    import jax
    import jax.numpy as jnp

    cpu = jax.devices("cpu")[0]
    with jax.default_device(cpu):
        xf = jnp.asarray(x.reshape(T, D), dtype=jnp.float32)
        wr = jnp.asarray(Wr, dtype=jnp.float32)
        probs = jax.nn.softmax(xf.astype(jnp.float32) @ wr, axis=-1)
        topk_probs, topk_experts = jax.lax.top_k(probs, TOP_K)
        idxs, gates = [], []
        for e in range(E):
            mask = topk_experts == e
            gate = jnp.sum(jnp.where(mask, topk_probs, 0.0), axis=-1)
            has = jnp.any(mask, axis=-1)
            g_masked = jnp.where(has, gate, -jnp.inf)
            vals, idx = jax.lax.top_k(g_masked, CAP)
            g = jnp.where(jnp.isfinite(vals), vals, 0.0)
            idxs.append(np.asarray(idx, dtype=np.int64))
            gates.append(np.asarray(g, dtype=np.float32))
    return idxs, gates


_COMPILED = None


def _build():
    """Build + compile the SPMD per-core program (identical on all 8 cores)."""
    import concourse.bacc as bacc
    import concourse.mybir as mybir
    import concourse.tile as tile

    f32 = mybir.dt.float32
    f32r = mybir.dt.float32r

    nc = bacc.Bacc("TRN2", target_bir_lowering=False, debug=False,
                   num_devices=NCORES)
    xg = nc.dram_tensor("xg", [D, 2 * CAP], f32r, kind="ExternalInput")
    w1 = nc.dram_tensor("w1", [2, D, FH], f32r, kind="ExternalInput")
    w2 = nc.dram_tensor("w2", [2, FH, D], f32r, kind="ExternalInput")
    ys = nc.dram_tensor("ys", [2, D, CAP], f32, kind="ExternalOutput")

    KD = D // P      # 8  k-chunks for matmul 1
    KF = FH // P     # 16 k-chunks for matmul 2
    with tile.TileContext(nc) as tc:
        with (
            tc.tile_pool(name="w1p", bufs=1) as w1p,
            tc.tile_pool(name="w2p", bufs=1) as w2p,
            tc.tile_pool(name="xgp", bufs=2) as xgp,
            tc.tile_pool(name="htp", bufs=1) as htp,
            tc.tile_pool(name="outp", bufs=2) as outp,
            tc.tile_pool(name="ps1", bufs=2, space="PSUM") as ps1,
            tc.tile_pool(name="ps2", bufs=2, space="PSUM") as ps2,
        ):
            for u in range(2):
                w1sb = []
                for k in range(KD):
                    t = w1p.tile([P, FH], f32r, tag=f"w1_{k}")
                    nc.sync.dma_start(t[:], w1[u, k * P:(k + 1) * P, :])
                    w1sb.append(t)
                w2sb = []
                for k in range(KF):
                    t = w2p.tile([P, D], f32r, tag=f"w2_{k}")
                    nc.sync.dma_start(t[:], w2[u, k * P:(k + 1) * P, :])
                    w2sb.append(t)
                for b in range(NBLK):
                    col0 = u * CAP + b * BLK
                    xgsb = []
                    for k in range(KD):
                        t = xgp.tile([P, BLK], f32r, tag=f"xg_{k}")
                        nc.sync.dma_start(
                            t[:], xg[k * P:(k + 1) * P, col0:col0 + BLK])
                        xgsb.append(t)
                    hts = []
                    for f in range(KF):
                        ps = ps1.tile([P, BLK], f32)
                        for k in range(KD):
                            nc.tensor.matmul(
                                ps[:], w1sb[k][:, f * P:(f + 1) * P], xgsb[k][:],
                                start=(k == 0), stop=(k == KD - 1))
                        ht = htp.tile([P, BLK], f32r, tag=f"ht_{f}")
                        nc.scalar.activation(
                            ht[:], ps[:], mybir.ActivationFunctionType.Relu)
                        hts.append(ht)
                    for d in range(KD):
                        ps_ = ps2.tile([P, BLK], f32)
                        for k2 in range(KF):
                            nc.tensor.matmul(
                                ps_[:], w2sb[k2][:, d * P:(d + 1) * P], hts[k2][:],
                                start=(k2 == 0), stop=(k2 == KF - 1))
                        ob = outp.tile([P, BLK], f32)
                        nc.vector.tensor_copy(ob[:], ps_[:])
                        nc.sync.dma_start(
                            ys[u, d * P:(d + 1) * P, b * BLK:(b + 1) * BLK],
                            ob[:])
    nc.compile()
    return nc


def _get_compiled():
    global _COMPILED
    if _COMPILED is None:
        _COMPILED = _build()
    return _COMPILED


def kernel(x, Wr, W1, W2, _timing=None):
    from concourse.bass_utils import run_bass_kernel_spmd

    x = np.asarray(x, dtype=np.float32)
    Wr = np.asarray(Wr, dtype=np.float32)
    W1 = np.asarray(W1, dtype=np.float32)
    W2 = np.asarray(W2, dtype=np.float32)
    xf = x.reshape(T, D)

    # --- Host router (replicated, reference-exact) => sharding plan ---
    idxs, gates = _route(xf, Wr)

    # --- Dispatch: gather + transpose routed rows per expert, tf32 grid ---
    xfT = _tf32_round(np.ascontiguousarray(xf.T))          # [D, T]
    in_maps = []
    for c in range(NCORES):
        j, h = c // 2, c % 2
        xg = np.concatenate(
            [xfT[:, idxs[2 * j]], xfT[:, idxs[2 * j + 1]]], axis=1)
        w1 = np.stack(
            [W1[2 * j][:, h * FH:(h + 1) * FH],
             W1[2 * j + 1][:, h * FH:(h + 1) * FH]])
        w2 = np.stack(
            [W2[2 * j][h * FH:(h + 1) * FH, :],
             W2[2 * j + 1][h * FH:(h + 1) * FH, :]])
        in_maps.append({
            "xg": np.ascontiguousarray(xg),
            "w1": _tf32_round(np.ascontiguousarray(w1)),
            "w2": _tf32_round(np.ascontiguousarray(w2)),
        })

    # --- Device: per-expert FFN halves on 8 cores ---
    nc = _get_compiled()
    res = run_bass_kernel_spmd(
        nc, in_maps, list(range(NCORES)),
        trace=(_timing is not None),
        trace_cores=list(range(NCORES)) if _timing is not None else None,
    )
    if _timing is not None:
        _timing["exec_time_ns"] = res.exec_time_ns
        _timing["results"] = res

    # --- Combine/unshard: sum halves, gate, scatter-add (host) ---
    y = np.zeros((T, D), dtype=np.float32)
    for j in range(NCORES // 2):
        ys0 = res.results[2 * j]["ys"]
        ys1 = res.results[2 * j + 1]["ys"]
        for u in range(2):
            e = 2 * j + u
            hs = ys0[u] + ys1[u]                           # [D, CAP]
            y[idxs[e]] += gates[e][:, None] * hs.T
    return y.reshape(B, S, D)


# revision 3
# speedup vs baseline: 1.2952x; 1.2952x over previous
"""MoE (8 experts, top-2, cap-drop) Trainium2 kernel over 8 NeuronCores.

Strategy (expert-parallel, per sharding hint):
 - Router runs replicated on host (tiny: 134 MFLOP of the 344 GFLOP total) with
   the exact fp32 jax ops of the reference so top-2/capacity decisions match
   the oracle bit-for-bit; routing IS the sharding function here - it decides
   which token rows go to which expert shard.
 - Dispatch/shard: per expert e, gather its routed token rows (ascending token
   order, gate 0 on padding slots) and ship them transposed (D on partitions).
 - Each expert's FFN is split into 2 "units" along the hidden axis (2048 units
   each), giving 16 units; each core runs 2 units sequentially, bin-packed by
   routed token count so all cores get equal block counts. Per unit:
       ysT = W2h^T-chain( relu( W1h^T-chain( xT ) ) )
   All matmuls in f32r (TF32 storage class, fp32 PSUM accumulate) at
   1 cycle/row on the PE array; host pre-rounds operands to the tf32 grid
   (measured exact on HW for tf32-representable inputs).
 - Combine/unshard: host sums the two half partial outputs per expert, applies
   the fp32 gates exactly as the reference (g[:,None] * h), and scatter-adds
   into y at the routed rows (row sets are disjoint within an expert).

Self-contained: shapes hardcoded for B=4, S=2048, D=1024, F=4096, E=8, top-2,
cap=2560, 8 cores.
"""

import sys

for _p in ("/opt/trn_rl_repo",):
    if _p not in sys.path:
        sys.path.append(_p)

import math

import numpy as np

B, S, D, F, E = 4, 2048, 1024, 4096, 8
TOP_K = 2
CAP_FACTOR = 1.25
T = B * S                                   # 8192 tokens
CAP = max(math.ceil(T * TOP_K * CAP_FACTOR / E), 1)   # 2560
FH = F // 2                                 # 2048 hidden units per core
NCORES = 8
BLK = 512                                   # token block (matmul moving dim)
P = 128


def _tf32_round(a: np.ndarray) -> np.ndarray:
    """Round-to-nearest-even fp32 -> tf32 (10-bit mantissa) on the host, so the
    on-device f32r matmuls see exactly-representable values (measured exact)."""
    u = a.reshape(-1).view(np.uint32).astype(np.uint64)
    r = (u + 0xFFF + ((u >> 13) & 1)) & ~np.uint64(0x1FFF)
    return r.astype(np.uint32).view(np.float32).reshape(a.shape)


def _route(xf: np.ndarray, Wr: np.ndarray):
    """Replicate the reference's routing bit-for-bit on jax-CPU.

    Returns per-expert (idx[CAP] int64 token ids, gate[CAP] f32, 0 on padding).
    """
    import jax
    import jax.numpy as jnp

    cpu = jax.devices("cpu")[0]
    with jax.default_device(cpu):
        xj = jnp.asarray(xf, dtype=jnp.float32)
        wr = jnp.asarray(Wr, dtype=jnp.float32)
        probs = jax.nn.softmax(xj.astype(jnp.float32) @ wr, axis=-1)
        topk_probs, topk_experts = jax.lax.top_k(probs, TOP_K)
        idxs, gates = [], []
        for e in range(E):
            mask = topk_experts == e
            gate = jnp.sum(jnp.where(mask, topk_probs, 0.0), axis=-1)
            has = jnp.any(mask, axis=-1)
            g_masked = jnp.where(has, gate, -jnp.inf)
            vals, idx = jax.lax.top_k(g_masked, CAP)
            g = jnp.where(jnp.isfinite(vals), vals, 0.0)
            idxs.append(np.asarray(idx, dtype=np.int64))
            gates.append(np.asarray(g, dtype=np.float32))
    return idxs, gates


_COMPILED = {}


def _build(nblk_a: int, nblk_b: int):
    """Compile the SPMD per-core program: two sequential units of a dense
    relu-MLP half, with nblk_a / nblk_b 512-token blocks respectively."""
    import concourse.bacc as bacc
    import concourse.mybir as mybir
    import concourse.tile as tile

    f32 = mybir.dt.float32
    f32r = mybir.dt.float32r

    nblks = (nblk_a, nblk_b)
    cols = [nblk_a * BLK, nblk_b * BLK]

    nc = bacc.Bacc("TRN2", target_bir_lowering=False, debug=False,
                   num_devices=NCORES)
    xg = nc.dram_tensor("xg", [D, sum(cols)], f32r, kind="ExternalInput")
    w1 = nc.dram_tensor("w1", [2, D, FH], f32r, kind="ExternalInput")
    w2 = nc.dram_tensor("w2", [2, FH, D], f32r, kind="ExternalInput")
    ysa = nc.dram_tensor("ysa", [D, cols[0]], f32, kind="ExternalOutput")
    ysb = nc.dram_tensor("ysb", [D, cols[1]], f32, kind="ExternalOutput")
    ys_ts = (ysa, ysb)

    KD = D // P      # 8  k-chunks for matmul 1
    KF = FH // P     # 16 k-chunks for matmul 2
    with tile.TileContext(nc) as tc:
        with (
            tc.tile_pool(name="w1p", bufs=1) as w1p,
            tc.tile_pool(name="w2p", bufs=1) as w2p,
            tc.tile_pool(name="xgp", bufs=2) as xgp,
            tc.tile_pool(name="htp", bufs=1) as htp,
            tc.tile_pool(name="outp", bufs=2) as outp,
            tc.tile_pool(name="ps1", bufs=2, space="PSUM") as ps1,
            tc.tile_pool(name="ps2", bufs=2, space="PSUM") as ps2,
        ):
            for u in range(2):
                w1sb = []
                for k in range(KD):
                    t = w1p.tile([P, FH], f32r, tag=f"w1_{k}")
                    nc.sync.dma_start(t[:], w1[u, k * P:(k + 1) * P, :])
                    w1sb.append(t)
                w2sb = [None] * KF
                col_base = u * cols[0]
                for b in range(nblks[u]):
                    col0 = col_base + b * BLK
                    xgsb = []
                    for k in range(KD):
                        t = xgp.tile([P, BLK], f32r, tag=f"xg_{k}")
                        nc.sync.dma_start(
                            t[:], xg[k * P:(k + 1) * P, col0:col0 + BLK])
                        xgsb.append(t)
                    hts = []
                    for f in range(KF):
                        ps = ps1.tile([P, BLK], f32)
                        for k in range(KD):
                            nc.tensor.matmul(
                                ps[:], w1sb[k][:, f * P:(f + 1) * P], xgsb[k][:],
                                start=(k == 0), stop=(k == KD - 1))
                        ht = htp.tile([P, BLK], f32r, tag=f"ht_{f}")
                        nc.scalar.activation(
                            ht[:], ps[:], mybir.ActivationFunctionType.Relu)
                        hts.append(ht)
                        if b == 0:
                            # Defer W2 streaming into m1 of the first block so
                            # startup only waits on W1 + first xg block.
                            t = w2p.tile([P, D], f32r, tag=f"w2_{f}")
                            nc.sync.dma_start(t[:], w2[u, f * P:(f + 1) * P, :])
                            w2sb[f] = t
                    for d in range(KD):
                        ps_ = ps2.tile([P, BLK], f32)
                        for k2 in range(KF):
                            nc.tensor.matmul(
                                ps_[:], w2sb[k2][:, d * P:(d + 1) * P], hts[k2][:],
                                start=(k2 == 0), stop=(k2 == KF - 1))
                        ob = outp.tile([P, BLK], f32)
                        nc.vector.tensor_copy(ob[:], ps_[:])
                        nc.sync.dma_start(
                            ys_ts[u][d * P:(d + 1) * P, b * BLK:(b + 1) * BLK],
                            ob[:])
    nc.compile()
    return nc


def _get_compiled(nblk_a: int, nblk_b: int):
    key = (nblk_a, nblk_b)
    if key not in _COMPILED:
        _COMPILED[key] = _build(*key)
    return _COMPILED[key]


def kernel(x, Wr, W1, W2, _timing=None):
    from concourse.bass_utils import run_bass_kernel_spmd

    x = np.asarray(x, dtype=np.float32)
    Wr = np.asarray(Wr, dtype=np.float32)
    W1 = np.asarray(W1, dtype=np.float32)
    W2 = np.asarray(W2, dtype=np.float32)
    xf = x.reshape(T, D)

    # --- Host router (replicated, reference-exact) => sharding plan ---
    idxs, gates = _route(xf, Wr)
    counts = [int(np.count_nonzero(gates[e])) for e in range(E)]
    sizes = [max(1, math.ceil(c / BLK)) for c in counts]  # blocks per expert

    # --- Bin-pack the 16 (expert, half) units onto 8 cores, 2 units each.
    # Units of one expert share its size; with exactly 8 small + 8 large (or
    # all equal) units, every core gets an identical (small, large) shape.
    units = [(e, h) for e in range(E) for h in range(2)]
    usz = {u: sizes[u[0]] for u in units}
    distinct = sorted(set(usz.values()))
    if len(distinct) == 1:
        nblk_a = nblk_b = distinct[0]
    elif (len(distinct) == 2
          and sum(1 for u in units if usz[u] == distinct[0]) == E):
        nblk_a, nblk_b = distinct
    else:
        nblk_a = nblk_b = max(distinct)      # fallback: uniform padding
    small = [u for u in units if usz[u] <= nblk_a][:E]
    large = [u for u in units if u not in small]
    assign = [(small[c], large[c]) for c in range(NCORES)]

    # --- Dispatch: gather + transpose routed rows per expert, tf32 grid ---
    xfT = _tf32_round(np.ascontiguousarray(xf.T))          # [D, T]
    cols = (nblk_a * BLK, nblk_b * BLK)
    in_maps = []
    for c in range(NCORES):
        parts_xg, parts_w1, parts_w2 = [], [], []
        for s, (e, h) in enumerate(assign[c]):
            parts_xg.append(xfT[:, idxs[e][:cols[s]]])
            parts_w1.append(W1[e][:, h * FH:(h + 1) * FH])
            parts_w2.append(W2[e][h * FH:(h + 1) * FH, :])
        in_maps.append({
            "xg": np.ascontiguousarray(np.concatenate(parts_xg, axis=1)),
            "w1": _tf32_round(np.ascontiguousarray(np.stack(parts_w1))),
            "w2": _tf32_round(np.ascontiguousarray(np.stack(parts_w2))),
        })

    # --- Device: 16 FFN half-units on 8 cores ---
    nc = _get_compiled(nblk_a, nblk_b)
    res = run_bass_kernel_spmd(
        nc, in_maps, list(range(NCORES)),
        trace=(_timing is not None),
        trace_cores=list(range(NCORES)) if _timing is not None else None,
    )
    if _timing is not None:
        _timing["exec_time_ns"] = res.exec_time_ns
        _timing["results"] = res

    # --- Combine/unshard: sum halves per expert, gate, scatter-add (host) ---
    part = {}                                  # (e, h) -> [cols, D]
    for c in range(NCORES):
        for s, (e, h) in enumerate(assign[c]):
            part[(e, h)] = res.results[c]["ysa" if s == 0 else "ysb"]
    y = np.zeros((T, D), dtype=np.float32)
    for e in range(E):
        hs = part[(e, 0)] + part[(e, 1)]       # [D, n_e]
        n = hs.shape[1]
        y[idxs[e][:n]] += gates[e][:n, None] * hs.T
    return y.reshape(B, S, D)


# revision 7
# speedup vs baseline: 1.3707x; 1.0583x over previous
"""MoE (8 experts, top-2, cap-drop) Trainium2 kernel over 8 NeuronCores.

Strategy (expert-parallel, per sharding hint):
 - Router runs replicated on host (tiny: 134 MFLOP of the 344 GFLOP total) with
   the exact fp32 jax ops of the reference so top-2/capacity decisions match
   the oracle bit-for-bit; routing IS the sharding function here - it decides
   which token rows go to which expert shard.
 - Dispatch/shard: per expert e, gather its routed token rows (ascending token
   order, gate 0 on padding slots) and ship them transposed (D on partitions).
 - Each expert's FFN is split into 2 "units" along the hidden axis (2048 units
   each), giving 16 units; each core runs 2 units sequentially, bin-packed by
   routed token count so all cores get equal block counts. Per unit:
       ysT = W2h^T-chain( relu( W1h^T-chain( xT ) ) )
   All matmuls in f32r (TF32 storage class, fp32 PSUM accumulate) at
   1 cycle/row on the PE array; host pre-rounds operands to the tf32 grid
   (measured exact on HW for tf32-representable inputs).
 - Combine/unshard: host sums the two half partial outputs per expert, applies
   the fp32 gates exactly as the reference (g[:,None] * h), and scatter-adds
   into y at the routed rows (row sets are disjoint within an expert).

Self-contained: shapes hardcoded for B=4, S=2048, D=1024, F=4096, E=8, top-2,
cap=2560, 8 cores.
"""

import sys

for _p in ("/opt/trn_rl_repo",):
    if _p not in sys.path:
        sys.path.append(_p)

import math

import numpy as np

B, S, D, F, E = 4, 2048, 1024, 4096, 8
TOP_K = 2
CAP_FACTOR = 1.25
T = B * S                                   # 8192 tokens
CAP = max(math.ceil(T * TOP_K * CAP_FACTOR / E), 1)   # 2560
FH = F // 2                                 # 2048 hidden units per core
NCORES = 8
BLK = 512                                   # token block (matmul moving dim)
P = 128


def _tf32_round(a: np.ndarray) -> np.ndarray:
    """Round-to-nearest-even fp32 -> tf32 (10-bit mantissa) on the host, so the
    on-device f32r matmuls see exactly-representable values (measured exact)."""
    u = a.reshape(-1).view(np.uint32).astype(np.uint64)
    r = (u + 0xFFF + ((u >> 13) & 1)) & ~np.uint64(0x1FFF)
    return r.astype(np.uint32).view(np.float32).reshape(a.shape)


def _route(xf: np.ndarray, Wr: np.ndarray):
    """Replicate the reference's routing bit-for-bit on jax-CPU.

    Returns per-expert (idx[CAP] int64 token ids, gate[CAP] f32, 0 on padding).
    """
    import jax
    import jax.numpy as jnp

    cpu = jax.devices("cpu")[0]
    with jax.default_device(cpu):
        xj = jnp.asarray(xf, dtype=jnp.float32)
        wr = jnp.asarray(Wr, dtype=jnp.float32)
        probs = jax.nn.softmax(xj.astype(jnp.float32) @ wr, axis=-1)
        topk_probs, topk_experts = jax.lax.top_k(probs, TOP_K)
        idxs, gates = [], []
        for e in range(E):
            mask = topk_experts == e
            gate = jnp.sum(jnp.where(mask, topk_probs, 0.0), axis=-1)
            has = jnp.any(mask, axis=-1)
            g_masked = jnp.where(has, gate, -jnp.inf)
            vals, idx = jax.lax.top_k(g_masked, CAP)
            g = jnp.where(jnp.isfinite(vals), vals, 0.0)
            idxs.append(np.asarray(idx, dtype=np.int64))
            gates.append(np.asarray(g, dtype=np.float32))
    return idxs, gates


_COMPILED = {}


def _build(nblk_a: int, nblk_b: int):
    """Compile the SPMD per-core program: two sequential units of a dense
    relu-MLP half, with nblk_a / nblk_b 512-token blocks respectively."""
    import concourse.bacc as bacc
    import concourse.mybir as mybir
    import concourse.tile as tile

    f32 = mybir.dt.float32
    f32r = mybir.dt.float32r

    nblks = (nblk_a, nblk_b)
    cols = [nblk_a * BLK, nblk_b * BLK]

    nc = bacc.Bacc("TRN2", target_bir_lowering=False, debug=False,
                   num_devices=NCORES)
    KD = D // P      # 8  k-chunks for matmul 1
    KF = FH // P     # 16 k-chunks for matmul 2
    xg = nc.dram_tensor("xg", [D, sum(cols)], f32r, kind="ExternalInput")
    # w1 host-pretiled f-major: [u, f, p, k*P+m] = W1h[k*P+p, f*P+m] so each
    # f-group is one contiguous 512 KB DMA and m1 can start after group 0.
    w1 = nc.dram_tensor("w1", [2, KF, P, D], f32r, kind="ExternalInput")
    w2 = nc.dram_tensor("w2", [2, FH, D], f32r, kind="ExternalInput")
    ysa = nc.dram_tensor("ysa", [D, cols[0]], f32, kind="ExternalOutput")
    ysb = nc.dram_tensor("ysb", [D, cols[1]], f32, kind="ExternalOutput")
    ys_ts = (ysa, ysb)

    with tile.TileContext(nc) as tc:
        with (
            tc.tile_pool(name="w1p", bufs=1) as w1p,
            tc.tile_pool(name="w2p", bufs=1) as w2p,
            tc.tile_pool(name="xgp", bufs=2) as xgp,
            tc.tile_pool(name="htp", bufs=1) as htp,
            tc.tile_pool(name="outp", bufs=2) as outp,
            tc.tile_pool(name="ps1", bufs=2, space="PSUM") as ps1,
            tc.tile_pool(name="ps2", bufs=2, space="PSUM") as ps2,
        ):
            for u in range(2):
                w1sb = [None] * KF
                w2sb = [None] * KF
                col_base = u * cols[0]
                for b in range(nblks[u]):
                    col0 = col_base + b * BLK
                    xgsb = []
                    for k in range(KD):
                        t = xgp.tile([P, BLK], f32r, tag=f"xg_{k}")
                        nc.sync.dma_start(
                            t[:], xg[k * P:(k + 1) * P, col0:col0 + BLK])
                        xgsb.append(t)
                    hts = []
                    for f in range(KF):
                        if b == 0:
                            # Stream W1/W2 f-groups at compute pace during the
                            # first block; resident for the rest of the unit.
                            t = w1p.tile([P, D], f32r, tag=f"w1_{f}")
                            nc.sync.dma_start(t[:], w1[u, f])
                            w1sb[f] = t
                        ps = ps1.tile([P, BLK], f32)
                        for k in range(KD):
                            nc.tensor.matmul(
                                ps[:], w1sb[f][:, k * P:(k + 1) * P], xgsb[k][:],
                                start=(k == 0), stop=(k == KD - 1))
                        ht = htp.tile([P, BLK], f32r, tag=f"ht_{f}")
                        nc.scalar.activation(
                            ht[:], ps[:], mybir.ActivationFunctionType.Relu)
                        hts.append(ht)
                        if b == 0:
                            t = w2p.tile([P, D], f32r, tag=f"w2_{f}")
                            nc.sync.dma_start(t[:], w2[u, f * P:(f + 1) * P, :])
                            w2sb[f] = t
                    for d in range(KD):
                        ps_ = ps2.tile([P, BLK], f32)
                        for k2 in range(KF):
                            nc.tensor.matmul(
                                ps_[:], w2sb[k2][:, d * P:(d + 1) * P], hts[k2][:],
                                start=(k2 == 0), stop=(k2 == KF - 1))
                        ob = outp.tile([P, BLK], f32)
                        nc.vector.tensor_copy(ob[:], ps_[:])
                        nc.sync.dma_start(
                            ys_ts[u][d * P:(d + 1) * P, b * BLK:(b + 1) * BLK],
                            ob[:])
    nc.compile()
    return nc


def _get_compiled(nblk_a: int, nblk_b: int):
    key = (nblk_a, nblk_b)
    if key not in _COMPILED:
        _COMPILED[key] = _build(*key)
    return _COMPILED[key]


def kernel(x, Wr, W1, W2, _timing=None):
    from concourse.bass_utils import run_bass_kernel_spmd

    x = np.asarray(x, dtype=np.float32)
    Wr = np.asarray(Wr, dtype=np.float32)
    W1 = np.asarray(W1, dtype=np.float32)
    W2 = np.asarray(W2, dtype=np.float32)
    xf = x.reshape(T, D)

    # --- Host router (replicated, reference-exact) => sharding plan ---
    idxs, gates = _route(xf, Wr)
    counts = [int(np.count_nonzero(gates[e])) for e in range(E)]
    sizes = [max(1, math.ceil(c / BLK)) for c in counts]  # blocks per expert

    # --- Bin-pack the 16 (expert, half) units onto 8 cores, 2 units each.
    # Units of one expert share its size; with exactly 8 small + 8 large (or
    # all equal) units, every core gets an identical (small, large) shape.
    units = [(e, h) for e in range(E) for h in range(2)]
    usz = {u: sizes[u[0]] for u in units}
    distinct = sorted(set(usz.values()))
    if len(distinct) == 1:
        nblk_a = nblk_b = distinct[0]
    elif (len(distinct) == 2
          and sum(1 for u in units if usz[u] == distinct[0]) == E):
        nblk_a, nblk_b = distinct
    else:
        nblk_a = nblk_b = max(distinct)      # fallback: uniform padding
    small = [u for u in units if usz[u] <= nblk_a][:E]
    large = [u for u in units if u not in small]
    assign = [(small[c], large[c]) for c in range(NCORES)]

    # --- Dispatch: gather + transpose routed rows per expert, tf32 grid ---
    xfT = _tf32_round(np.ascontiguousarray(xf.T))          # [D, T]
    cols = (nblk_a * BLK, nblk_b * BLK)
    in_maps = []
    for c in range(NCORES):
        parts_xg, parts_w1, parts_w2 = [], [], []
        for s, (e, h) in enumerate(assign[c]):
            parts_xg.append(xfT[:, idxs[e][:cols[s]]])
            # f-major pretile: [f, p, k*P+m] = W1h[k*P+p, f*P+m]
            w1h = W1[e][:, h * FH:(h + 1) * FH]
            w1t = w1h.reshape(D // P, P, FH // P, P).transpose(2, 1, 0, 3)
            parts_w1.append(w1t.reshape(FH // P, P, D))
            parts_w2.append(W2[e][h * FH:(h + 1) * FH, :])
        in_maps.append({
            "xg": np.ascontiguousarray(np.concatenate(parts_xg, axis=1)),
            "w1": _tf32_round(np.ascontiguousarray(np.stack(parts_w1))),
            "w2": _tf32_round(np.ascontiguousarray(np.stack(parts_w2))),
        })

    # --- Device: 16 FFN half-units on 8 cores ---
    nc = _get_compiled(nblk_a, nblk_b)
    res = run_bass_kernel_spmd(
        nc, in_maps, list(range(NCORES)),
        trace=(_timing is not None),
        trace_cores=list(range(NCORES)) if _timing is not None else None,
    )
    if _timing is not None:
        _timing["exec_time_ns"] = res.exec_time_ns
        _timing["results"] = res

    # --- Combine/unshard: sum halves per expert, gate, scatter-add (host) ---
    part = {}                                  # (e, h) -> [cols, D]
    for c in range(NCORES):
        for s, (e, h) in enumerate(assign[c]):
            part[(e, h)] = res.results[c]["ysa" if s == 0 else "ysb"]
    y = np.zeros((T, D), dtype=np.float32)
    for e in range(E):
        hs = part[(e, 0)] + part[(e, 1)]       # [D, n_e]
        n = hs.shape[1]
        y[idxs[e][:n]] += gates[e][:n, None] * hs.T
    return y.reshape(B, S, D)
